# revision 37
# baseline (speedup 1.0000x reference)
"""Trainium2 Bass kernel for an 8-head MultiHeadAttention (B=2, S=4096, H=512).

Sharding: 8 NeuronCores, each takes (one batch, two heads):
    core c -> batch b = c // 4, heads {2*(c%4), 2*(c%4)+1}.

Optimized pipeline (vs. the 358us baseline):
  - Projections run in bf16 (x and W pre-cast on host): halves the input
    DMA (4MB/core) and keeps PE at 1 cycle/row.  Wave order k -> q -> v so
    scores can start as soon as q's first block is evicted.
  - The attention inner loop is software-pipelined with a lookahead of
    LOOK kc-chunks so the PE never waits on the softmax exp:
        scores(kc) -> exp(kc) [split across engines] -> attn@v(kc-LOOK)
  - The exp of each [128, 2*512] score tile is split across THREE engines
    working concurrently on disjoint column ranges: Act (true Exp), DVE
    and GpSimd (Schraudolph bit-trick exp: bf16 bits = int16(s*A + B),
    +-3.3% multiplicative, validated ~1.3e-2 scale-relative absmax).
  - Normalization: denominator row (from the ones-column in v) is copied
    to f32r, broadcast across 64 partitions by a K=1 ones-matmul,
    reciprocal_approx_fast + multiply on DVE, bf16 output (host upcasts).
"""

import sys

sys.path.insert(0, "/opt/trn_rl_repo")

import ml_dtypes
import numpy as np

import concourse.bass as bass  # noqa: E402
import concourse.tile as tile  # noqa: E402
from concourse import bacc, mybir  # noqa: E402
from concourse.bass_utils import run_bass_kernel_spmd  # noqa: E402

B, S, H = 2, 4096, 512
NH, HD = 8, 64
NCORES = 8
HPC = 2  # heads per core
DPC = HPC * HD  # head dims per core = 128
P = 128  # partitions
QB = 512  # query block (matmul free dim)
KC = 128  # key chunk (contraction tile)
KF = H // P  # feature chunks for projections = 4
NKC = S // KC  # 32
NQB = S // QB  # 8
VPAD = 80  # padded per-(kc,h) v row (64 v + ones + align padding)
LOOK = 2  # attn@v lookahead (kc chunks) for PE pipelining

# Per-head exp engines: Act owns head 0, DVE (Schraudolph) owns head 1.
# GpSimd cannot read PSUM, so only these two engines can evict scores.
LOOK_H = (2, 3)  # attn@v lookahead per head (h1 later: DVE exp is slower)

# Schraudolph constants: bf16(exp(s/8)) bits ~= int16(s*EXP_A + EXP_B)
SCALE = 1.0 / np.sqrt(HD)
EXP_A = float(128.0 * np.log2(np.e) * SCALE)
EXP_B = float(128.0 * (127.0 - 0.0436))

f32 = mybir.dt.float32
f32r = mybir.dt.float32r
bf16 = mybir.dt.bfloat16
i16 = mybir.dt.int16
_np_bf16 = ml_dtypes.bfloat16


def _emit_kernel(ctx, tc, outT, xT, wq, wk, wv, bias3, ones, onescol):
    nc = tc.nc

    const = ctx.enter_context(tc.tile_pool(name="const", bufs=1))

    # ---- DMA order: xT chunk 0 first (the k-wave gates on it), then
    # weights, remaining chunks interleaved.  Weights come pre-arranged
    # from the host as [128, KF*128] so each DMA is contiguous/partition.
    wq_sb = const.tile([P, KF, DPC], bf16)
    wk_sb = const.tile([P, KF, DPC], bf16)
    wv_sb = const.tile([P, KF, DPC], bf16)
    bias_sb = const.tile([P, 3], f32)
    xT_sb = const.tile([P, KF, S], bf16)

    # Input DMAs are spread across the three DMA-capable queues (SP,
    # Activation, GpSimd): a single queue moves only ~25 GB/s in 4KB
    # packets, which serialized the whole 4MB input behind one ring.
    queues = [nc.sync, nc.scalar]

    def xchunk(q, kf, hh, quarters=1):
        w = S // 2 // quarters
        for i in range(quarters):
            c0 = hh * (S // 2) + i * w
            q.dma_start(
                out=xT_sb[:, kf, c0 : c0 + w],
                in_=xT[kf * P : (kf + 1) * P, c0 : c0 + w],
            )

    # wk sliced by kf (the first k matmuls need only the kf0 slice) and
    # the first xT chunk in sb-sized slices: the first matmul fires after
    # ~96KB instead of the full 640KB
    nc.scalar.dma_start(out=wk_sb[:, 0, :], in_=wk[:, 0:DPC])
    # biases [3, 128] -> sbuf [128, 3] (partition = output dim; q, k, v)
    nc.scalar.dma_start(out=bias_sb[:], in_=bias3.rearrange("a m -> m a"))
    xchunk(nc.sync, 0, 0, quarters=4)
    nc.scalar.dma_start(
        out=wk_sb[:, 1:KF, :].rearrange("p a b -> p (a b)"),
        in_=wk[:, DPC:],
    )
    xchunk(nc.scalar, 0, 1, quarters=4)
    nc.scalar.dma_start(out=wq_sb.rearrange("p a b -> p (a b)"), in_=wq[:])
    nc.sync.dma_start(out=wv_sb.rearrange("p a b -> p (a b)"), in_=wv[:])
    xchunk(nc.sync, 1, 0)
    xchunk(nc.scalar, 1, 1)
    xchunk(nc.sync, 2, 0)
    xchunk(nc.scalar, 2, 1)
    xchunk(nc.sync, 3, 0)
    xchunk(nc.scalar, 3, 1)

    # ---- projections: q/k/v in T layout, bf16 matmuls, bf16 evictions ----
    # k is stored zero-PADDED to K=128 per head (kp_sb[:, h]: head h's
    # 64 dims on its own partition range, zeros elsewhere) so the score
    # matmuls run in the same (128, 128) PE tiling mode as attn@v --
    # avoiding a TensorE drain on every mode switch.
    qT_sb = const.tile([P, S], bf16)
    kp_sb = const.tile([P, 2, S], bf16)
    nc.vector.memset(kp_sb[HD:P, 0, :], 0.0)
    nc.vector.memset(kp_sb[0:HD, 1, :], 0.0)
    vT_sb = const.tile([P, S], bf16)
    # v natural + ones column: vp_sb[p, kc, h, :64] = v, [..., 64] = 1
    vp_sb = const.tile([P, NKC, HPC, VPAD], bf16)
    ones_sb = const.tile([1, P], f32r)
    nc.sync.dma_start(out=ones_sb[:], in_=ones[:])
    nc.sync.dma_start(out=vp_sb[:, :, :, HD : HD + 1], in_=onescol[:])

    with tc.tile_pool(name="proj_psum", bufs=8, space="PSUM") as pp:
        with nc.named_scope("proj"):
            # wave order: k, q (so scores can start), then v
            for proj, w_sb in ((1, wk_sb), (0, wq_sb), (2, wv_sb)):
                pss = [
                    pp.tile([P, QB], f32, tag="ps", name=f"pj{proj}_{sb}")
                    for sb in range(S // QB)
                ]
                # kf-outer: the first 8 matmuls need only xT chunk 0
                for kf in range(KF):
                    for sb in range(S // QB):
                        nc.tensor.matmul(
                            pss[sb][:],
                            lhsT=w_sb[:, kf, :],
                            rhs=xT_sb[:, kf, sb * QB : (sb + 1) * QB],
                            start=(kf == 0),
                            stop=(kf == KF - 1),
                        )
                for sb in range(S // QB):
                    s0, s1 = sb * QB, (sb + 1) * QB
                    # psum -> sbuf eviction, fused bias add, bf16 out
                    with nc.allow_low_precision(reason="bf16 attention"):
                        if proj == 1:  # k: two per-head padded evictions
                            for h in range(HPC):
                                rows = slice(h * HD, (h + 1) * HD)
                                nc.vector.tensor_scalar_add(
                                    kp_sb[rows, h, s0:s1],
                                    pss[sb][rows, :],
                                    bias_sb[rows, proj : proj + 1],
                                )
                        else:
                            dst = (
                                vT_sb[:, s0:s1]
                                if proj == 2
                                else qT_sb[:, s0:s1]
                            )
                            nc.vector.tensor_scalar_add(
                                dst, pss[sb][:], bias_sb[:, proj : proj + 1]
                            )
                if proj == 2:
                    # v: T layout -> natural via hardware DMA transpose
                    # (X-bar, bf16), one per head: in [64, S] -> out
                    # [128, NKC, 64].  Keep both on ONE queue and unsplit:
                    # concurrent/pipelined X-bar transposes corrupt.
                    for h in range(HPC):
                        nc.sync.dma_start_transpose(
                            out=vp_sb[:, :, h, 0:HD],
                            in_=vT_sb[h * HD : (h + 1) * HD, :],
                        )

    # ---- attention ----
    # PSUM budget (8 banks): sch 5 + oT0 2 + oT1 1 (h0's accumulator is
    # double-buffered across query blocks to hide its normalization)
    sc_pool = ctx.enter_context(tc.tile_pool(name="sc", bufs=5, space="PSUM"))
    ot_pool = ctx.enter_context(tc.tile_pool(name="ot", bufs=1, space="PSUM"))
    ex_pool = ctx.enter_context(tc.tile_pool(name="ex", bufs=5))
    rc_pool = ctx.enter_context(tc.tile_pool(name="rc", bufs=4))
    res_pool = ctx.enter_context(tc.tile_pool(name="res", bufs=4))

    with nc.named_scope("attn"):
        for qb in range(NQB):
            q0, q1 = qb * QB, (qb + 1) * QB
            look = LOOK_H
            oT = [
                ot_pool.tile(
                    [HD + 1, QB],
                    f32,
                    tag=f"oT{h}",
                    name=f"oT{qb}_{h}",
                    bufs=2 if h == 0 else 1,
                )
                for h in range(HPC)
            ]
            ex_tiles = {}

            def attnv(kc, h):
                nc.tensor.matmul(
                    oT[h][:],
                    lhsT=vp_sb[:, kc, h, 0 : HD + 1],
                    rhs=ex_tiles[kc][h][:],
                    start=(kc == 0),
                    stop=(kc == NKC - 1),
                )

            for kc in range(NKC):
                # per-head 1-bank score tiles -> deeper recycle slack
                scs = [
                    sc_pool.tile([P, QB], f32, tag="sch", name=f"sc{qb}_{kc}_{h}")
                    for h in range(HPC)
                ]
                for h in range(HPC):
                    # scoresT[k, q] for head h; K = 128 via the zero-padded
                    # k tile -> same PE tiling mode as attn@v (no drains)
                    nc.tensor.matmul(
                        scs[h][:],
                        lhsT=kp_sb[:, h, kc * KC : (kc + 1) * KC],
                        rhs=qT_sb[:, q0:q1],
                        start=True,
                        stop=True,
                    )
                # separate per-head ex tiles: a shared tile would create a
                # false WAW between the two engines' writes
                exA = ex_pool.tile([P, QB], bf16, tag="exA", name=f"exA{qb}_{kc}")
                exB = ex_pool.tile([P, QB], bf16, tag="exB", name=f"exB{qb}_{kc}")
                ex_tiles[kc] = (exA, exB)
                # exp: Act engine evicts head 0 (true Exp), DVE head 1
                # (Schraudolph bit-trick) -- both straight from PSUM
                nc.scalar.activation(
                    exA[:],
                    scs[0][:],
                    mybir.ActivationFunctionType.Exp,
                    scale=SCALE,
                )
                with nc.allow_low_precision(reason="schraudolph exp"):
                    nc.vector.tensor_scalar(
                        exB[:].bitcast(i16),
                        scs[1][:],
                        EXP_A,
                        EXP_B,
                        mybir.AluOpType.mult,
                        mybir.AluOpType.add,
                    )
                for h in range(HPC):
                    if kc >= look[h]:
                        attnv(kc - look[h], h)
            for h in range(HPC):
                for kc in range(NKC - look[h], NKC):
                    attnv(kc, h)

            for h in range(HPC):
                # denominator row -> sbuf, reciprocal, then broadcast to 64
                # partitions on GpSimd (keeps the PE out of normalization)
                srow = rc_pool.tile([1, QB], f32, tag="srow", name=f"sr{qb}_{h}")
                nc.vector.tensor_copy(srow[:], oT[h][HD : HD + 1, :])
                rsr = rc_pool.tile([1, QB], f32, tag="rsr", name=f"rs{qb}_{h}")
                nc.vector.reciprocal_approx_fast(out=rsr[:], in_=srow[:])
                rcb = res_pool.tile([HD, QB], f32, tag="rcb", name=f"rcb{qb}_{h}")
                nc.gpsimd.partition_broadcast(rcb[:], rsr[:])
                res = res_pool.tile([HD, QB], bf16, tag="res")
                with nc.allow_low_precision(reason="bf16 output"):
                    nc.vector.tensor_mul(res[:], oT[h][:HD, :], rcb[:])
                nc.sync.dma_start(
                    out=outT[h * HD : (h + 1) * HD, q0:q1], in_=res[:]
                )


def build_nc():
    from contextlib import ExitStack

    nc = bacc.Bacc(
        "TRN2",
        target_bir_lowering=False,
        debug=False,
        num_devices=NCORES,
    )
    xT = nc.dram_tensor("xT", [H, S], bf16, kind="ExternalInput").ap()
    # weights pre-arranged on host to [128, KF*128] (partition-contiguous)
    wq = nc.dram_tensor("wq", [P, KF * DPC], bf16, kind="ExternalInput").ap()
    wk = nc.dram_tensor("wk", [P, KF * DPC], bf16, kind="ExternalInput").ap()
    wv = nc.dram_tensor("wv", [P, KF * DPC], bf16, kind="ExternalInput").ap()
    bias3 = nc.dram_tensor("bias3", [3, DPC], f32, kind="ExternalInput").ap()
    ones = nc.dram_tensor("ones", [1, P], f32r, kind="ExternalInput").ap()
    onescol = nc.dram_tensor(
        "onescol", [P, NKC * HPC], bf16, kind="ExternalInput"
    ).ap()
    outT = nc.dram_tensor("outT", [DPC, S], bf16, kind="ExternalOutput").ap()
    with tile.TileContext(nc) as tc, ExitStack() as ctx:
        _emit_kernel(ctx, tc, outT, xT, wq, wk, wv, bias3, ones, onescol)
    nc.compile()
    return nc


_NC_CACHE = None


def _get_nc():
    global _NC_CACHE
    if _NC_CACHE is None:
        _NC_CACHE = build_nc()
    return _NC_CACHE


def _shard_inputs(x, Wq, bq, Wk, bk, Wv, bv):
    """Build per-core input maps (host does layout only: transpose/slice)."""
    x = np.ascontiguousarray(np.asarray(x, dtype=np.float32))
    in_maps = []
    xT_by_batch = [np.ascontiguousarray(x[b].T).astype(_np_bf16) for b in range(B)]

    def warr(W, cols):
        # [512, 128] -> [128 (p), KF*128] so the device DMA is contiguous
        w = np.asarray(W, np.float32)[:, cols].astype(_np_bf16)
        return np.ascontiguousarray(
            w.reshape(KF, P, DPC).transpose(1, 0, 2).reshape(P, KF * DPC)
        )

    for c in range(NCORES):
        b, p = c // (NCORES // B), c % (NCORES // B)
        cols = slice(p * DPC, (p + 1) * DPC)
        in_maps.append(
            {
                "xT": xT_by_batch[b],
                "wq": warr(Wq, cols),
                "wk": warr(Wk, cols),
                "wv": warr(Wv, cols),
                "ones": np.ones((1, P), dtype=np.float32),
                "onescol": np.ones((P, NKC * HPC), dtype=_np_bf16),
                "bias3": np.stack(
                    [
                        np.asarray(bq, np.float32)[cols],
                        np.asarray(bk, np.float32)[cols],
                        np.asarray(bv, np.float32)[cols],
                    ]
                ),
            }
        )
    return in_maps


def _assemble(results):
    out = np.empty((B, S, H), dtype=np.float32)
    for c in range(NCORES):
        b, p = c // (NCORES // B), c % (NCORES // B)
        outT = results[c]["outT"]  # [128, S] bf16
        out[b, :, p * DPC : (p + 1) * DPC] = outT.astype(np.float32).T
    return out


def run(inputs, trace=False):
    nc = _get_nc()
    in_maps = _shard_inputs(**inputs)
    res = run_bass_kernel_spmd(nc, in_maps, list(range(NCORES)), trace=trace)
    return _assemble(res.results), res


def kernel(**inputs):
    out, _ = run(inputs)
    return out


# revision 39
# speedup vs baseline: 1.0019x; 1.0019x over previous
"""Trainium2 Bass kernel for an 8-head MultiHeadAttention (B=2, S=4096, H=512).

Sharding: 8 NeuronCores, each takes (one batch, two heads):
    core c -> batch b = c // 4, heads {2*(c%4), 2*(c%4)+1}.

Optimized pipeline (vs. the 358us baseline):
  - Projections run in bf16 (x and W pre-cast on host): halves the input
    DMA (4MB/core) and keeps PE at 1 cycle/row.  Wave order k -> q -> v so
    scores can start as soon as q's first block is evicted.
  - The attention inner loop is software-pipelined with a lookahead of
    LOOK kc-chunks so the PE never waits on the softmax exp:
        scores(kc) -> exp(kc) [split across engines] -> attn@v(kc-LOOK)
  - The exp of each [128, 2*512] score tile is split across THREE engines
    working concurrently on disjoint column ranges: Act (true Exp), DVE
    and GpSimd (Schraudolph bit-trick exp: bf16 bits = int16(s*A + B),
    +-3.3% multiplicative, validated ~1.3e-2 scale-relative absmax).
  - Normalization: denominator row (from the ones-column in v) is copied
    to f32r, broadcast across 64 partitions by a K=1 ones-matmul,
    reciprocal_approx_fast + multiply on DVE, bf16 output (host upcasts).
"""

import sys

sys.path.insert(0, "/opt/trn_rl_repo")

import ml_dtypes
import numpy as np

import concourse.bass as bass  # noqa: E402
import concourse.tile as tile  # noqa: E402
from concourse import bacc, mybir  # noqa: E402
from concourse.bass_utils import run_bass_kernel_spmd  # noqa: E402

B, S, H = 2, 4096, 512
NH, HD = 8, 64
NCORES = 8
HPC = 2  # heads per core
DPC = HPC * HD  # head dims per core = 128
P = 128  # partitions
QB = 512  # query block (matmul free dim)
KC = 128  # key chunk (contraction tile)
KF = H // P  # feature chunks for projections = 4
NKC = S // KC  # 32
NQB = S // QB  # 8
VPAD = 80  # padded per-(kc,h) v row (64 v + ones + align padding)
LOOK = 2  # attn@v lookahead (kc chunks) for PE pipelining

# Per-head exp engines: Act owns head 0, DVE (Schraudolph) owns head 1.
# GpSimd cannot read PSUM, so only these two engines can evict scores.
LOOK_H = (2, 3)  # attn@v lookahead per head (h1 later: DVE exp is slower)

# Schraudolph constants: bf16(exp(s/8)) bits ~= int16(s*EXP_A + EXP_B)
SCALE = 1.0 / np.sqrt(HD)
EXP_A = float(128.0 * np.log2(np.e) * SCALE)
EXP_B = float(128.0 * (127.0 - 0.0436))

f32 = mybir.dt.float32
f32r = mybir.dt.float32r
bf16 = mybir.dt.bfloat16
i16 = mybir.dt.int16
_np_bf16 = ml_dtypes.bfloat16


def _emit_kernel(ctx, tc, outT, xT, wq, wk, wv, bias3, ones, onescol):
    nc = tc.nc

    const = ctx.enter_context(tc.tile_pool(name="const", bufs=1))

    # ---- DMA order: xT chunk 0 first (the k-wave gates on it), then
    # weights, remaining chunks interleaved.  Weights come pre-arranged
    # from the host as [128, KF*128] so each DMA is contiguous/partition.
    wq_sb = const.tile([P, KF, DPC], bf16)
    wk_sb = const.tile([P, KF, DPC], bf16)
    wv_sb = const.tile([P, KF, DPC], bf16)
    bias_sb = const.tile([P, 3], f32)
    xT_sb = const.tile([P, KF, S], bf16)

    # Input DMAs are spread across the three DMA-capable queues (SP,
    # Activation, GpSimd): a single queue moves only ~25 GB/s in 4KB
    # packets, which serialized the whole 4MB input behind one ring.
    queues = [nc.sync, nc.scalar]

    def xchunk(q, kf, hh, quarters=1):
        w = S // 2 // quarters
        for i in range(quarters):
            c0 = hh * (S // 2) + i * w
            q.dma_start(
                out=xT_sb[:, kf, c0 : c0 + w],
                in_=xT[kf * P : (kf + 1) * P, c0 : c0 + w],
            )

    # wk sliced by kf (the first k matmuls need only the kf0 slice) and
    # the first xT chunk in sb-sized slices: the first matmul fires after
    # ~96KB instead of the full 640KB
    nc.scalar.dma_start(out=wk_sb[:, 0, :], in_=wk[:, 0:DPC])
    # biases [3, 128] -> sbuf [128, 3] (partition = output dim; q, k, v)
    nc.scalar.dma_start(out=bias_sb[:], in_=bias3.rearrange("a m -> m a"))
    xchunk(nc.sync, 0, 0, quarters=4)
    nc.scalar.dma_start(
        out=wk_sb[:, 1:KF, :].rearrange("p a b -> p (a b)"),
        in_=wk[:, DPC:],
    )
    xchunk(nc.scalar, 0, 1, quarters=4)
    nc.scalar.dma_start(out=wq_sb.rearrange("p a b -> p (a b)"), in_=wq[:])
    nc.sync.dma_start(out=wv_sb.rearrange("p a b -> p (a b)"), in_=wv[:])
    xchunk(nc.sync, 1, 0)
    xchunk(nc.scalar, 1, 1)
    xchunk(nc.sync, 2, 0)
    xchunk(nc.scalar, 2, 1)
    xchunk(nc.sync, 3, 0)
    xchunk(nc.scalar, 3, 1)

    # ---- projections: q/k/v in T layout, bf16 matmuls, bf16 evictions ----
    # k is stored zero-PADDED to K=128 per head (kp_sb[:, h]: head h's
    # 64 dims on its own partition range, zeros elsewhere) so the score
    # matmuls run in the same (128, 128) PE tiling mode as attn@v --
    # avoiding a TensorE drain on every mode switch.
    qT_sb = const.tile([P, S], bf16)
    kp_sb = const.tile([P, 2, S], bf16)
    nc.vector.memset(kp_sb[HD:P, 0, :], 0.0)
    nc.vector.memset(kp_sb[0:HD, 1, :], 0.0)
    vT_sb = const.tile([P, S], bf16)
    # v natural + ones column: vp_sb[p, kc, h, :64] = v, [..., 64] = 1
    vp_sb = const.tile([P, NKC, HPC, VPAD], bf16)
    ones_sb = const.tile([1, P], f32r)
    nc.sync.dma_start(out=ones_sb[:], in_=ones[:])
    nc.sync.dma_start(out=vp_sb[:, :, :, HD : HD + 1], in_=onescol[:])

    with tc.tile_pool(name="proj_psum", bufs=8, space="PSUM") as pp:
        with nc.named_scope("proj"):
            # wave order: k, q (so scores can start), then v
            for proj, w_sb in ((1, wk_sb), (0, wq_sb), (2, wv_sb)):
                pss = [
                    pp.tile([P, QB], f32, tag="ps", name=f"pj{proj}_{sb}")
                    for sb in range(S // QB)
                ]
                # kf-outer: the first 8 matmuls need only xT chunk 0
                for kf in range(KF):
                    for sb in range(S // QB):
                        nc.tensor.matmul(
                            pss[sb][:],
                            lhsT=w_sb[:, kf, :],
                            rhs=xT_sb[:, kf, sb * QB : (sb + 1) * QB],
                            start=(kf == 0),
                            stop=(kf == KF - 1),
                        )
                for sb in range(S // QB):
                    s0, s1 = sb * QB, (sb + 1) * QB
                    # psum -> sbuf eviction, fused bias add, bf16 out
                    with nc.allow_low_precision(reason="bf16 attention"):
                        if proj == 1:  # k: two per-head padded evictions
                            for h in range(HPC):
                                rows = slice(h * HD, (h + 1) * HD)
                                nc.vector.tensor_scalar_add(
                                    kp_sb[rows, h, s0:s1],
                                    pss[sb][rows, :],
                                    bias_sb[rows, proj : proj + 1],
                                )
                        else:
                            dst = (
                                vT_sb[:, s0:s1]
                                if proj == 2
                                else qT_sb[:, s0:s1]
                            )
                            nc.vector.tensor_scalar_add(
                                dst, pss[sb][:], bias_sb[:, proj : proj + 1]
                            )
                if proj == 2:
                    # v: T layout -> natural via hardware DMA transpose
                    # (X-bar, bf16), one per head: in [64, S] -> out
                    # [128, NKC, 64].  Keep both on ONE queue and unsplit:
                    # concurrent/pipelined X-bar transposes corrupt.
                    for h in range(HPC):
                        nc.sync.dma_start_transpose(
                            out=vp_sb[:, :, h, 0:HD],
                            in_=vT_sb[h * HD : (h + 1) * HD, :],
                        )

    # ---- attention ----
    # PSUM budget (8 banks): sch 5 + oT0 2 + oT1 1 (h0's accumulator is
    # double-buffered across query blocks to hide its normalization)
    sc_pool = ctx.enter_context(tc.tile_pool(name="sc", bufs=5, space="PSUM"))
    ot_pool = ctx.enter_context(tc.tile_pool(name="ot", bufs=1, space="PSUM"))
    ex_pool = ctx.enter_context(tc.tile_pool(name="ex", bufs=5))
    rc_pool = ctx.enter_context(tc.tile_pool(name="rc", bufs=4))
    res_pool = ctx.enter_context(tc.tile_pool(name="res", bufs=4))

    with nc.named_scope("attn"):
        for qb in range(NQB):
            q0, q1 = qb * QB, (qb + 1) * QB
            look = LOOK_H
            oT = [
                ot_pool.tile(
                    [HD + 1, QB],
                    f32,
                    tag=f"oT{h}",
                    name=f"oT{qb}_{h}",
                    bufs=2 if h == 0 else 1,
                )
                for h in range(HPC)
            ]
            ex_tiles = {}

            def attnv(kc, h):
                nc.tensor.matmul(
                    oT[h][:],
                    lhsT=vp_sb[:, kc, h, 0 : HD + 1],
                    rhs=ex_tiles[kc][h][:],
                    start=(kc == 0),
                    stop=(kc == NKC - 1),
                )

            for kc in range(NKC):
                # per-head 1-bank score tiles -> deeper recycle slack
                scs = [
                    sc_pool.tile([P, QB], f32, tag="sch", name=f"sc{qb}_{kc}_{h}")
                    for h in range(HPC)
                ]
                for h in range(HPC):
                    # scoresT[k, q] for head h; K = 128 via the zero-padded
                    # k tile -> same PE tiling mode as attn@v (no drains)
                    nc.tensor.matmul(
                        scs[h][:],
                        lhsT=kp_sb[:, h, kc * KC : (kc + 1) * KC],
                        rhs=qT_sb[:, q0:q1],
                        start=True,
                        stop=True,
                    )
                # separate per-head ex tiles: a shared tile would create a
                # false WAW between the two engines' writes
                exA = ex_pool.tile([P, QB], bf16, tag="exA", name=f"exA{qb}_{kc}")
                exB = ex_pool.tile([P, QB], bf16, tag="exB", name=f"exB{qb}_{kc}")
                ex_tiles[kc] = (exA, exB)
                # exp: Act engine evicts head 0 (true Exp), DVE head 1
                # (Schraudolph bit-trick) -- both straight from PSUM
                nc.scalar.activation(
                    exA[:],
                    scs[0][:],
                    mybir.ActivationFunctionType.Exp,
                    scale=SCALE,
                )
                with nc.allow_low_precision(reason="schraudolph exp"):
                    nc.vector.tensor_scalar(
                        exB[:].bitcast(i16),
                        scs[1][:],
                        EXP_A,
                        EXP_B,
                        mybir.AluOpType.mult,
                        mybir.AluOpType.add,
                    )
                for h in range(HPC):
                    if kc >= look[h]:
                        attnv(kc - look[h], h)
            for h in range(HPC):
                for kc in range(NKC - look[h], NKC):
                    attnv(kc, h)

            for h in range(HPC):
                # denominator row -> sbuf, reciprocal, then broadcast to 64
                # partitions on GpSimd (keeps the PE out of normalization)
                srow = rc_pool.tile([1, QB], f32, tag="srow", name=f"sr{qb}_{h}")
                nc.vector.tensor_copy(srow[:], oT[h][HD : HD + 1, :])
                rsr = rc_pool.tile([1, QB], f32, tag="rsr", name=f"rs{qb}_{h}")
                nc.vector.reciprocal_approx_fast(out=rsr[:], in_=srow[:])
                rcb = res_pool.tile([HD, QB], f32, tag="rcb", name=f"rcb{qb}_{h}")
                nc.gpsimd.partition_broadcast(rcb[:], rsr[:])
                res = res_pool.tile([HD, QB], bf16, tag="res")
                with nc.allow_low_precision(reason="bf16 output"):
                    nc.vector.tensor_mul(res[:], oT[h][:HD, :], rcb[:])
                nc.sync.dma_start(
                    out=outT[h * HD : (h + 1) * HD, q0:q1], in_=res[:]
                )


def build_nc():
    from contextlib import ExitStack

    nc = bacc.Bacc(
        "TRN2",
        target_bir_lowering=False,
        debug=False,
        num_devices=NCORES,
    )
    xT = nc.dram_tensor("xT", [H, S], bf16, kind="ExternalInput").ap()
    # weights pre-arranged on host to [128, KF*128] (partition-contiguous)
    wq = nc.dram_tensor("wq", [P, KF * DPC], bf16, kind="ExternalInput").ap()
    wk = nc.dram_tensor("wk", [P, KF * DPC], bf16, kind="ExternalInput").ap()
    wv = nc.dram_tensor("wv", [P, KF * DPC], bf16, kind="ExternalInput").ap()
    bias3 = nc.dram_tensor("bias3", [3, DPC], f32, kind="ExternalInput").ap()
    ones = nc.dram_tensor("ones", [1, P], f32r, kind="ExternalInput").ap()
    onescol = nc.dram_tensor(
        "onescol", [P, NKC * HPC], bf16, kind="ExternalInput"
    ).ap()
    outT = nc.dram_tensor("outT", [DPC, S], bf16, kind="ExternalOutput").ap()
    with tile.TileContext(nc) as tc, ExitStack() as ctx:
        _emit_kernel(ctx, tc, outT, xT, wq, wk, wv, bias3, ones, onescol)
    nc.compile()
    return nc


_NC_CACHE = None


def _get_nc():
    global _NC_CACHE
    if _NC_CACHE is None:
        _NC_CACHE = build_nc()
    return _NC_CACHE


def _shard_inputs(x, Wq, bq, Wk, bk, Wv, bv):
    """Build per-core input maps (host does layout only: transpose/slice)."""
    x = np.ascontiguousarray(np.asarray(x, dtype=np.float32))
    in_maps = []
    xT_by_batch = [np.ascontiguousarray(x[b].T).astype(_np_bf16) for b in range(B)]

    def warr(W, cols):
        # [512, 128] -> [128 (p), KF*128] so the device DMA is contiguous
        w = np.asarray(W, np.float32)[:, cols].astype(_np_bf16)
        return np.ascontiguousarray(
            w.reshape(KF, P, DPC).transpose(1, 0, 2).reshape(P, KF * DPC)
        )

    for c in range(NCORES):
        b, p = c // (NCORES // B), c % (NCORES // B)
        cols = slice(p * DPC, (p + 1) * DPC)
        in_maps.append(
            {
                "xT": xT_by_batch[b],
                "wq": warr(Wq, cols),
                "wk": warr(Wk, cols),
                "wv": warr(Wv, cols),
                "ones": np.ones((1, P), dtype=np.float32),
                "onescol": np.ones((P, NKC * HPC), dtype=_np_bf16),
                "bias3": np.stack(
                    [
                        np.asarray(bq, np.float32)[cols],
                        np.asarray(bk, np.float32)[cols],
                        np.asarray(bv, np.float32)[cols],
                    ]
                ),
            }
        )
    return in_maps


def _assemble(results):
    out = np.empty((B, S, H), dtype=np.float32)
    for c in range(NCORES):
        b, p = c // (NCORES // B), c % (NCORES // B)
        outT = results[c]["outT"]  # [128, S] bf16
        out[b, :, p * DPC : (p + 1) * DPC] = outT.astype(np.float32).T
    return out


def run(inputs, trace=False):
    nc = _get_nc()
    in_maps = _shard_inputs(**inputs)
    res = run_bass_kernel_spmd(nc, in_maps, list(range(NCORES)), trace=trace)
    return _assemble(res.results), res


def kernel(**inputs):
    out, _ = run(inputs)
    return out
